# revision 11
# baseline (speedup 1.0000x reference)
"""CrossAttention (channel attention) Trainium2 kernel, v2.

Math (per batch element b):
    q = x Wq^T ; k = y Wk^T ; v = y Wv^T          (N=4096 tokens, C=1024 ch)
    per head h (H=16, D=64):
      scores_h = (Qh^T Kh) * D^-0.5 = Wq_h (x^T y) Wk_h^T * s   (D x D)
      attn_h = softmax(scores_h, axis=-1)
      z_h    = Vh attn_h^T                         (N x D)
    out = z Wp^T + bp

Reassociated (saves ~40% FLOPs and avoids transposing x):
    G   = y^T x                    (C x C)   contraction over n: natural layouts
    A   = G^T Wk^T                 (C x C)
    S_h = (s*Wq_h) A_h             (D x D)  -> softmax (unnormalized probs P_h,
                                              row sums r)
    M_h = P_h Wv_h                 (D x C);  Mall[ci, h*D+d] = M_h[d, ci]/r_d
    P   = Mall Wp^T                (C x C)
    out = y P + bp                 (N x C)

Sharding: pure data-parallel over batch B=8 across the 8 NeuronCores.
All on-chip matmuls run in fp16 (full PE rate) with fp32 PSUM accumulation.

v2 changes vs v1:
  - y^T is pre-transposed on host (yt16 input) -> linear DMA instead of the
    descriptor-heavy on-device DMA transpose.
  - x is kept fully resident in SBUF (4 subtiles); phase-1 second pass
    re-streams from SBUF instead of re-DMAing. The subtiles are tag-recycled
    for wqts/wv/wpt/osb after phase 1 (those DMAs dispatch from the scalar
    queue so their WAR waits don't block the sync queue).
  - phase 1 is split pass A (3 column tiles, nt-outer, overlaps input DMA)
    + pass B (5 column tiles, cj-outer) on one 4-slot rotating PSUM pool,
    so PSUM->SBUF copies overlap following matmuls instead of bunching.
  - softmax phase is batched: all score matmuls, then transposes, then
    M-matmuls, with bufs=8 small pools so engine queues never cross-block.
  - ~3.8us of dummy warmup matmuls issued during the framework preamble so
    HAM un-throttles before the first real matmul.
  - last output tile's bias-add + store is split in two to shorten the tail.
"""

import sys

import numpy as np

sys.path.insert(0, "/opt/trn_rl_repo")

import concourse.bass as bass  # noqa: E402
import concourse.mybir as mybir  # noqa: E402
import concourse.tile as tile  # noqa: E402
from concourse import bacc  # noqa: E402
from concourse.masks import make_identity  # noqa: E402

F16 = mybir.dt.float16
F32 = mybir.dt.float32
AX = mybir.AxisListType
AF = mybir.ActivationFunctionType

B, N, C, H = 8, 4096, 1024, 16
D = C // H          # 64
SCALE = D ** -0.5
NT = N // 128       # 32 n-tiles
CT = C // 128       # 8 channel tiles
PAIRS = H // 2      # 8 head pairs
PASS_A = 3          # G column tiles computed in pass A (nt-outer)


def build_kernel():
    nc = bacc.Bacc("TRN2", target_bir_lowering=False)

    x_d = nc.dram_tensor("x16", [N, C], F16, kind="ExternalInput")
    y_d = nc.dram_tensor("y16", [N, C], F16, kind="ExternalInput")
    yt_d = nc.dram_tensor("yt16", [C, N], F16, kind="ExternalInput")
    wqts_d = nc.dram_tensor("wqts", [C, C], F16, kind="ExternalInput")  # (Wq*s).T
    wkt_d = nc.dram_tensor("wkt", [C, C], F16, kind="ExternalInput")    # Wk.T
    wv_d = nc.dram_tensor("wv", [C, C], F16, kind="ExternalInput")      # Wv
    wpt_d = nc.dram_tensor("wpt", [C, C], F16, kind="ExternalInput")    # Wp.T
    bp_d = nc.dram_tensor("bp", [C], F32, kind="ExternalInput")
    out_d = nc.dram_tensor("out", [N, C], F32, kind="ExternalOutput")

    with tile.TileContext(nc) as tc:
        with (
            tc.tile_pool(name="persist", bufs=1) as persist,
            tc.tile_pool(name="small", bufs=8) as small,
        ):
            # ---------------- HAM warmup ------------------------------
            # PE sits idle for ~4us of framework preamble + first DMAs;
            # dummy matmuls on a zeroed tile keep it busy so the clock
            # gate opens (1.2 -> 2.4 GHz) before the first real matmul.
            scratch = persist.tile([128, 128], F16, name="scratch")
            nc.gpsimd.memset(scratch, 0.0)
            with tc.tile_pool(name="ps_w", bufs=1, space="PSUM") as ps_w_pool:
                ps_warm = ps_w_pool.tile([128, 128], F32, name="ps_warm")
                for _ in range(36):
                    nc.tensor.matmul(ps_warm, lhsT=scratch, rhs=scratch,
                                     start=True, stop=True)

            # ---------------- input DMAs (sync queue) -----------------
            # y16: stationary operand of phase 1.  x16: resident streaming
            # operand, 4 subtiles whose slots are recycled after phase 1.
            y16 = persist.tile([128, NT, C], F16, name="y16", tag="ybig")
            xs = [persist.tile([128, 8, C], F16, name=f"xs{j}", tag=f"xs{j}")
                  for j in range(4)]
            for nt in range(NT):
                nc.sync.dma_start(y16[:, nt, :], y_d[nt * 128:(nt + 1) * 128, :])
                nc.sync.dma_start(xs[nt // 8][:, nt % 8, :],
                                  x_d[nt * 128:(nt + 1) * 128, :])

            wkt = persist.tile([128, CT, C], F16, name="wkt_sb")
            nc.sync.dma_start(wkt, wkt_d[:].rearrange("(t p) c -> p t c", p=128))

            bias = persist.tile([128, C], F32, name="bias_sb")
            bp_ap = bp_d[:]
            nc.sync.dma_start(
                bias,
                bass.AP(tensor=bp_ap.tensor, offset=bp_ap.offset,
                        ap=[[0, 128]] + list(bp_ap.ap)),
            )

            id128 = persist.tile([128, 128], F16, name="id128")
            make_identity(nc, id128)
            # identity block living on partitions 64..127: idhi[64+i, i] = 1
            idhi = persist.tile([128, D], F16, name="idhi")
            nc.gpsimd.memset(idhi, 0.0)
            nc.gpsimd.affine_select(
                out=idhi, in_=idhi,
                compare_op=mybir.AluOpType.not_equal,
                fill=1.0, base=-D, pattern=[[-1, D]], channel_multiplier=1,
            )

            # ================= phase 1: G = y^T x =======================
            # One rotating 4-slot PSUM pool (8 banks).  Pass A computes G
            # column-tiles 0..2 nt-outer (slots 0-2) while x/y stream in;
            # pass B computes tiles 3..7 cj-outer from resident x, each
            # copy overlapping the next tile's matmuls.
            g2 = persist.tile([128, CT, C], F16, name="g2_sb", tag="sc1")
            with tc.tile_pool(name="ps_g", bufs=4, space="PSUM") as ps_g_pool:
                psA = [ps_g_pool.tile([128, C], F32, name=f"ps_gA{cj}",
                                      tag="psg") for cj in range(PASS_A)]
                for nt in range(NT):
                    xt = xs[nt // 8][:, nt % 8, :]
                    for cj in range(PASS_A):
                        for ch in range(2):
                            nc.tensor.matmul(
                                psA[cj][:, ch * 512:(ch + 1) * 512],
                                lhsT=y16[:, nt, cj * 128:(cj + 1) * 128],
                                rhs=xt[:, ch * 512:(ch + 1) * 512],
                                start=(nt == 0), stop=(nt == NT - 1),
                            )
                for cj in range(PASS_A):
                    nc.vector.tensor_copy(out=g2[:, cj, 0:512],
                                          in_=psA[cj][:, 0:512])
                    nc.scalar.activation(out=g2[:, cj, 512:1024],
                                         in_=psA[cj][:, 512:1024], func=AF.Copy)

                for cj in range(PASS_A, CT):
                    psB = ps_g_pool.tile([128, C], F32, name=f"ps_gB{cj}",
                                         tag="psg")
                    for nt in range(NT):
                        xt = xs[nt // 8][:, nt % 8, :]
                        for ch in range(2):
                            nc.tensor.matmul(
                                psB[:, ch * 512:(ch + 1) * 512],
                                lhsT=y16[:, nt, cj * 128:(cj + 1) * 128],
                                rhs=xt[:, ch * 512:(ch + 1) * 512],
                                start=(nt == 0), stop=(nt == NT - 1),
                            )
                    nc.vector.tensor_copy(out=g2[:, cj, 0:512],
                                          in_=psB[:, 0:512])
                    nc.scalar.activation(out=g2[:, cj, 512:1024],
                                         in_=psB[:, 512:1024], func=AF.Copy)

            # -------- late DMAs (scalar queue; WAR-wait on x/y slots) ---
            # These overwrite the x subtiles / y16 slot, so their waits
            # would block the sync queue; the scalar engine is idle here.
            wqts = persist.tile([128, CT, C], F16, name="wqts_sb", tag="xs0")
            wv = persist.tile([128, CT, C], F16, name="wv_sb", tag="xs1")
            wpt = persist.tile([128, CT, C], F16, name="wpt_sb", tag="xs2")
            for sb, dr in ((wqts, wqts_d), (wv, wv_d), (wpt, wpt_d)):
                nc.gpsimd.dma_start(sb, dr[:].rearrange("(t p) c -> p t c", p=128))
            ytall = persist.tile([128, CT, N], F16, name="ytall", tag="ybig")
            for k in range(CT):
                nc.gpsimd.dma_start(
                    ytall[:, k, :], yt_d[k * 128:(k + 1) * 128, :])
            # output staging reuses xs3's slot ([128,4,1024] f32 == 16KB)
            osb = persist.tile([128, 4, C], F32, name="osb", tag="xs3")

            # ================= phase 3: A = G^T Wk^T ====================
            a_sb = persist.tile([128, CT, C], F16, name="a_sb", tag="sc2")
            with tc.tile_pool(name="ps_a", bufs=2, space="PSUM") as ps_a_pool:
                for ci in range(CT):
                    psa = ps_a_pool.tile([128, C], F32, name="ps_a")
                    for cj in range(CT):
                        for ch in range(2):
                            nc.tensor.matmul(
                                psa[:, ch * 512:(ch + 1) * 512],
                                lhsT=g2[:, cj, ci * 128:(ci + 1) * 128],
                                rhs=wkt[:, cj, ch * 512:(ch + 1) * 512],
                                start=(cj == 0), stop=(cj == CT - 1),
                            )
                    nc.vector.tensor_copy(out=a_sb[:, ci, 0:512],
                                          in_=psa[:, 0:512])
                    nc.scalar.activation(out=a_sb[:, ci, 512:1024],
                                         in_=psa[:, 512:1024], func=AF.Copy)

            # ====== phase 4+5: scores -> softmax -> Mall^T ==============
            # Batched: all score matmuls first (PE stays dense), softmax
            # chains ride behind on Vector/Scalar, then transposes, then
            # M-matmuls.  Small pools are 8 deep so no slot is reused
            # within the phase (avoids FIFO cross-waits).
            mallT = persist.tile([128, CT, C], F16, name="mallT", tag="sc1")
            probs = [None] * PAIRS
            rcp = [None] * PAIRS
            attnT = [None] * PAIRS
            # ps_t/ps_m created first so the allocator places them below
            # ps_s's banks -- a transpose must not WAR-wait on a score bank.
            with (
                tc.tile_pool(name="ps_t", bufs=2, space="PSUM") as ps_t_pool,
                tc.tile_pool(name="ps_m", bufs=2, space="PSUM") as ps_m_pool,
                tc.tile_pool(name="ps_s", bufs=4, space="PSUM") as ps_s_pool,
            ):
                for t in range(PAIRS):
                    ps_s = ps_s_pool.tile([128, D], F32, name="ps_s")
                    for h2 in range(2):
                        h = 2 * t + h2
                        hsl = slice(h * D, (h + 1) * D)
                        for ci in range(CT):
                            nc.tensor.matmul(
                                ps_s[h2 * D:(h2 + 1) * D, :],
                                lhsT=wqts[:, ci, hsl],
                                rhs=a_sb[:, ci, hsl],
                                start=(ci == 0), stop=(ci == CT - 1),
                            )
                    mx = small.tile([128, 1], F32, name="mx")
                    nc.vector.reduce_max(out=mx, in_=ps_s, axis=AX.X, negate=True)
                    probs[t] = small.tile([128, D], F16, name="probs")
                    sumex = small.tile([128, 1], F32, name="sumex")
                    nc.scalar.activation(
                        out=probs[t], in_=ps_s, func=AF.Exp,
                        bias=mx, scale=1.0, accum_out=sumex,
                    )
                    rcp[t] = small.tile([128, 1], F32, name="rcp")
                    nc.vector.reciprocal(out=rcp[t], in_=sumex)

                for t in range(PAIRS):
                    at_ps = ps_t_pool.tile([128, D], F16, name="at_ps")
                    nc.tensor.transpose(at_ps[0:D, :], probs[t][0:D, :],
                                        id128[0:D, 0:D])
                    nc.tensor.transpose(at_ps[D:128, :], probs[t][D:128, :],
                                        idhi[D:128, :])
                    attnT[t] = small.tile([128, D], F16, name="attnT")
                    nc.vector.tensor_copy(out=attnT[t], in_=at_ps)

                for t in range(PAIRS):
                    for ch in range(2):
                        csl = slice(ch * 512, (ch + 1) * 512)
                        ps_m = ps_m_pool.tile([128, 512], F32, name="ps_m")
                        nc.tensor.matmul(ps_m[0:D, :], lhsT=attnT[t][0:D, :],
                                         rhs=wv[0:D, t, csl],
                                         start=True, stop=True)
                        nc.tensor.matmul(ps_m[D:128, :], lhsT=attnT[t][D:128, :],
                                         rhs=wv[D:128, t, csl],
                                         start=True, stop=True)
                        nc.vector.tensor_scalar_mul(
                            out=mallT[:, t, csl], in0=ps_m, scalar1=rcp[t],
                        )

            # ================= phase 6: P = Mall Wp^T ===================
            p_sb = persist.tile([128, CT, C], F16, name="p_sb", tag="sc2")
            with tc.tile_pool(name="ps_p", bufs=2, space="PSUM") as ps_p_pool:
                for ci in range(CT):
                    psp = ps_p_pool.tile([128, C], F32, name="ps_p")
                    for cp in range(CT):
                        for ch in range(2):
                            nc.tensor.matmul(
                                psp[:, ch * 512:(ch + 1) * 512],
                                lhsT=mallT[:, cp, ci * 128:(ci + 1) * 128],
                                rhs=wpt[:, cp, ch * 512:(ch + 1) * 512],
                                start=(cp == 0), stop=(cp == CT - 1),
                            )
                    nc.vector.tensor_copy(out=p_sb[:, ci, 0:512],
                                          in_=psp[:, 0:512])
                    nc.scalar.activation(out=p_sb[:, ci, 512:1024],
                                         in_=psp[:, 512:1024], func=AF.Copy)

            # ================= phase 7: out = y P + bp ==================
            with tc.tile_pool(name="ps_f", bufs=3, space="PSUM") as ps_f_pool:
                for nt in range(NT):
                    psf = ps_f_pool.tile([128, C], F32, name="ps_f")
                    ob = osb[:, nt % 4, :]
                    row = slice(nt * 128, (nt + 1) * 128)
                    if nt < NT - 1:
                        for k in range(CT):
                            for ch in range(2):
                                nc.tensor.matmul(
                                    psf[:, ch * 512:(ch + 1) * 512],
                                    lhsT=ytall[:, k, nt * 128:(nt + 1) * 128],
                                    rhs=p_sb[:, k, ch * 512:(ch + 1) * 512],
                                    start=(k == 0), stop=(k == CT - 1),
                                )
                        nc.vector.tensor_add(out=ob, in0=psf, in1=bias)
                        nc.sync.dma_start(out_d[row, :], ob)
                    else:
                        # last tile: ch-outer so the first half's add+store
                        # overlaps the second half's matmuls -> shorter tail
                        for ch in range(2):
                            csl = slice(ch * 512, (ch + 1) * 512)
                            for k in range(CT):
                                nc.tensor.matmul(
                                    psf[:, csl],
                                    lhsT=ytall[:, k, nt * 128:(nt + 1) * 128],
                                    rhs=p_sb[:, k, csl],
                                    start=(k == 0), stop=(k == CT - 1),
                                )
                            nc.vector.tensor_add(out=ob[:, csl], in0=psf[:, csl],
                                                 in1=bias[:, csl])
                            nc.sync.dma_start(out_d[row, csl], ob[:, csl])

    nc.compile()
    return nc


_NC_CACHE = None


def _get_nc():
    global _NC_CACHE
    if _NC_CACHE is None:
        _NC_CACHE = build_kernel()
    return _NC_CACHE


def run(inputs, trace=False, **kw):
    from concourse.bass_utils import run_bass_kernel_spmd

    x = np.asarray(inputs["x"], dtype=np.float32)
    y = np.asarray(inputs["y"], dtype=np.float32)
    Wq = np.asarray(inputs["Wq"], dtype=np.float32)
    Wk = np.asarray(inputs["Wk"], dtype=np.float32)
    Wv = np.asarray(inputs["Wv"], dtype=np.float32)
    Wp = np.asarray(inputs["Wp"], dtype=np.float32)
    bp = np.asarray(inputs["bp"], dtype=np.float32)

    wqts = np.ascontiguousarray((Wq.T * np.float32(SCALE)).astype(np.float16))
    wkt = np.ascontiguousarray(Wk.T.astype(np.float16))
    wv16 = np.ascontiguousarray(Wv.astype(np.float16))
    wpt = np.ascontiguousarray(Wp.T.astype(np.float16))

    x16 = x.astype(np.float16)
    y16 = y.astype(np.float16)

    nc = _get_nc()
    in_maps = [
        {
            "x16": np.ascontiguousarray(x16[b]),
            "y16": np.ascontiguousarray(y16[b]),
            "yt16": np.ascontiguousarray(y16[b].T),
            "wqts": wqts,
            "wkt": wkt,
            "wv": wv16,
            "wpt": wpt,
            "bp": bp,
        }
        for b in range(B)
    ]
    res = run_bass_kernel_spmd(nc, in_maps, core_ids=list(range(B)),
                               trace=trace, **kw)
    out = np.stack([res.results[b]["out"] for b in range(B)], axis=0)
    return out, res


def kernel(**inputs) -> np.ndarray:
    out, _ = run(inputs)
    return out


if __name__ == "__main__":
    nc = build_kernel()
    print("build ok")
